# revision 10
# baseline (speedup 1.0000x reference)
"""Trainium2 Bass kernel for nn_MAS (3-layer GAT-style attention product).

Math:
    for l in 0..2:
        Wh  = X @ Ws[l].T + bWs[l]
        e_ij = leaky_relu(f1_i + f2_j + ba[l]),  f1 = Wh@a1[l], f2 = Wh@a2[l]
        alpha = softmax_row(e);  A_MAS *= alpha
    out = A_MAS @ X

Key identities:
  * f1 = X @ (Ws.T a1) + bWs.a1 -> host computes per-layer vectors
    A_l[i] (= f1_i + consts) and B_l[j] (= f2_j) in float64.
  * prod_l softmax(e_l) = exp(sum_l leaky(z_l)) / prod_l S_l with
    z_l[i,j] = A_l[i] + B_l[j].  Row sums S_l are host-computable in
    O(N log N) by sorting B_l, so the device computes only
    YT = P^T-layout @ X with P = exp(sum_l leaky(z_l)) and the host applies
    the 1/prod_l S_l row scaling (plus the final transpose).

Device ([j, i] transposed layout -> no on-chip transposes):
  * PE: z_l tiles [128 j, 512 i] as K=8 bf16 outer-sum matmuls into PSUM
    (bf16 hi+lo splits of A/B make z exact to ~fp32).
  * ACT: Prelu(alpha=0.2) == exact leaky;  DVE: fused max(z,.2z)+m custom op;
    one Exp (ACT) since the product of exps is the exp of the summed leakys.
  * PE: YT[64, 512] accumulated over the 64 j-tiles, X in bf16 hi+lo splits.
Each of 8 cores handles 1024 i-rows; X replicated; no collectives.
"""

import sys

sys.path.insert(0, "/opt/trn_rl_repo")

from contextlib import ExitStack

import numpy as np
import ml_dtypes

BF16 = ml_dtypes.bfloat16

N = 8192
D = 64
L = 3
ALPHA = 0.2
N_CORES = 8
ROWS = N // N_CORES          # 1024 i-rows per core
CHUNK = 512                  # i-extent per chunk
NCHUNK = ROWS // CHUNK       # 2
NB = N // 128                # 64 j-tiles
GRP = 2                      # j-tiles per staging group
NGRP = NB // GRP             # 32

# Tuning knobs (read at _build_nc time).
CONF = {
    "use_custom_dve": True,
    # out of every 8 groups, this many use the DVE-heavy variant
    # (ACT: Prelu(z0)+Exp; DVE: leaky_add(z1), leaky_add(z2)); the rest use
    # (ACT: Prelu(z0), Prelu(z1), Exp; DVE/Pool: add, leaky_add(z2)).
    "beta8": 8,
    "pool_add": False,  # non-beta groups: do the m+t add on GPSIMD
    "mbufs": 4,
    "pbufs": 4,
    "zbufs": 3,
}

_CACHED = {}


def _get_leaky_add():
    """Register (once) the fused DVE op  out = max(in0, alpha*in0) + in1."""
    if "leaky_add" in _CACHED:
        return _CACHED["leaky_add"]
    from concourse import dve_ops as dvo
    from concourse.dve_spec import Spec, Src0, Src1, C2, maxx, lower
    from concourse.dve_uop import DveOpSpec

    name = "LEAKY_ADD_MAS"
    spec = Spec(
        body=maxx(Src0, Src0 * C2) + Src1,
        reference=lambda in0, in1, imm2: np.maximum(in0, in0 * imm2) + in1,
    )
    shas = {}
    for ver in ("v3", "v4"):
        tmp = DveOpSpec(name=name, opcode=0, uops=lower(spec, ver=ver),
                        rd1_en=True)
        shas[ver] = tmp.sha(ver)
    op = dvo.DveOp(name, spec, subdim=False, uops_sha=shas)
    if name not in dvo._SUB_OPCODE_FOR_NAME:
        dvo.OPS.append(op)
        row = dvo._CUSTOM_DVE_ROW_BASE + len(dvo.OPS) - 1
        assert row < 0x20
        dvo._SUB_OPCODE_FOR_NAME[name] = row
    _CACHED["leaky_add"] = op
    return op


def _build_nc(reps: int = 1, dyn_loop: bool = False):
    import concourse.bass as bass
    import concourse.tile as tile
    from concourse import bacc, mybir

    nc = bacc.Bacc("TRN2", target_bir_lowering=False, debug=False,
                   num_devices=N_CORES)
    f32 = mybir.dt.float32
    bf16 = mybir.dt.bfloat16
    AF = mybir.ActivationFunctionType
    leaky_add = _get_leaky_add() if CONF["use_custom_dve"] else None
    if dyn_loop:
        nit_d = nc.dram_tensor("nit", [1, 1], mybir.dt.int32,
                               kind="ExternalInput")

    # K=8 outer-sum operands: lhsT rows (B0h,B0l,B1h,B1l,B2h,B2l,1,1);
    # rhs rows for layer l: rows 2l,2l+1 = ones, row 6/7 = Ah_l/Al_l, rest 0.
    bigW_d = nc.dram_tensor("bigW", [8, N], bf16, kind="ExternalInput")
    xh_d = nc.dram_tensor("xh", [128, NB * D], bf16, kind="ExternalInput")
    xl_d = nc.dram_tensor("xl", [128, NB * D], bf16, kind="ExternalInput")
    rhsA_d = nc.dram_tensor("rhsA", [8, L * NCHUNK * CHUNK], bf16,
                            kind="ExternalInput")
    yt_d = nc.dram_tensor("yt", [D, ROWS], f32, kind="ExternalOutput")

    with tile.TileContext(nc) as tc:
        with ExitStack() as ctx:
            consts = ctx.enter_context(tc.tile_pool(name="consts", bufs=1))
            zpool = ctx.enter_context(
                tc.tile_pool(name="zpool", bufs=CONF["zbufs"], space="PSUM"))
            opool = ctx.enter_context(
                tc.tile_pool(name="opool", bufs=2, space="PSUM"))
            mpool = ctx.enter_context(tc.tile_pool(name="mpool", bufs=CONF["mbufs"]))
            ppool = ctx.enter_context(tc.tile_pool(name="ppool", bufs=CONF["pbufs"]))

            sbW = consts.tile([8, N], bf16)
            nc.sync.dma_start(sbW[:], bigW_d.ap()[:])
            sbA = consts.tile([8, L * NCHUNK * CHUNK], bf16)
            nc.sync.dma_start(sbA[:], rhsA_d.ap()[:])
            sbXh = consts.tile([128, NB * D], bf16)
            nc.gpsimd.dma_start(sbXh[:], xh_d.ap()[:])
            sbXl = consts.tile([128, NB * D], bf16)
            nc.gpsimd.dma_start(sbXl[:], xl_d.ap()[:])
            ysb = consts.tile([D, ROWS], f32)

            GW = GRP * CHUNK  # free-dim columns per staged group

            def emit_tail(pso, p, g):
                """Exp + final matmuls for group g (deferred one group for
                ACT/PE pipelining)."""
                for bi in range(GRP):
                    b = g * GRP + bi
                    rhs = p[:, bi * CHUNK:(bi + 1) * CHUNK]
                    nc.tensor.matmul(
                        pso[:], sbXh[:, b * D:(b + 1) * D], rhs,
                        start=(b == 0), stop=False,
                        skip_group_check=True)
                    nc.tensor.matmul(
                        pso[:], sbXl[:, b * D:(b + 1) * D], rhs,
                        start=False, stop=(b == NB - 1),
                        skip_group_check=True)

            def emit_body(_rep):
                for c in range(NCHUNK):
                    pso = opool.tile([D, CHUNK], f32, name=f"pso_{_rep}_{c}",
                                     tag="pso")
                    pending = None
                    for g in range(NGRP):
                        beta = CONF["use_custom_dve"] and \
                            (g % 8) < CONF["beta8"]
                        zs = []
                        for l in range(L):
                            zt = zpool.tile([128, GW], f32,
                                            name=f"z_{_rep}_{c}_{g}_{l}",
                                            tag="z")
                            for bi in range(GRP):
                                b = g * GRP + bi
                                nc.tensor.matmul(
                                    zt[:, bi * CHUNK:(bi + 1) * CHUNK],
                                    sbW[:, b * 128:(b + 1) * 128],
                                    sbA[:, (l * NCHUNK + c) * CHUNK:
                                        (l * NCHUNK + c + 1) * CHUNK],
                                    start=True, stop=True)
                            zs.append(zt)
                        m = mpool.tile([128, GW], f32,
                                       name=f"m_{_rep}_{c}_{g}", tag="m")
                        nc.scalar.activation(m[:], zs[0][:], AF.Prelu,
                                             bias=0.0, scale=1.0, alpha=ALPHA)
                        if beta:
                            for l in (1, 2):
                                nc.vector._custom_dve(
                                    leaky_add, out=m[:], in0=zs[l][:],
                                    in1=m[:], imm2=ALPHA)
                        else:
                            t = mpool.tile([128, GW], f32,
                                           name=f"t_{_rep}_{c}_{g}", tag="t")
                            nc.scalar.activation(t[:], zs[1][:], AF.Prelu,
                                                 bias=0.0, scale=1.0,
                                                 alpha=ALPHA)
                            if CONF["pool_add"]:
                                nc.gpsimd.tensor_add(m[:], m[:], t[:])
                            else:
                                nc.vector.tensor_add(m[:], m[:], t[:])
                            if CONF["use_custom_dve"]:
                                nc.vector._custom_dve(
                                    leaky_add, out=m[:], in0=zs[2][:],
                                    in1=m[:], imm2=ALPHA)
                            else:
                                t2 = mpool.tile([128, GW], f32,
                                                name=f"t2_{_rep}_{c}_{g}",
                                                tag="t")
                                nc.scalar.activation(t2[:], zs[2][:],
                                                     AF.Prelu, bias=0.0,
                                                     scale=1.0, alpha=ALPHA)
                                nc.vector.tensor_add(m[:], m[:], t2[:])
                        if pending is not None:
                            pm, pg = pending
                            pp = ppool.tile([128, GW], bf16,
                                            name=f"p_{_rep}_{c}_{pg}", tag="p")
                            nc.scalar.activation(pp[:], pm[:], AF.Exp)
                            emit_tail(pso, pp, pg)
                        pending = (m, g)
                    pm, pg = pending
                    pp = ppool.tile([128, GW], bf16,
                                    name=f"p_{_rep}_{c}_{pg}", tag="p")
                    nc.scalar.activation(pp[:], pm[:], AF.Exp)
                    emit_tail(pso, pp, pg)
                    nc.scalar.copy(ysb[:, c * CHUNK:(c + 1) * CHUNK], pso[:])

            if dyn_loop:
                nit_sb = consts.tile([1, 1], mybir.dt.int32)
                nc.sync.dma_start(nit_sb[:], nit_d.ap()[:])
                nit = nc.values_load(
                    nit_sb[0:1, 0:1].to_broadcast((1, 1)))
                with tc.For_i(0, nit, 1,
                              hint_engines=(mybir.EngineType.PE,
                                            mybir.EngineType.Activation,
                                            mybir.EngineType.DVE)):
                    emit_body("dyn")
            else:
                for _rep in range(reps):
                    emit_body(_rep)
            nc.sync.dma_start(yt_d.ap()[:], ysb[:])
    nc.finalize()
    return nc


def _split_bf16(a):
    hi = a.astype(BF16)
    lo = (a - hi.astype(np.float64)).astype(BF16)
    return hi, lo


def _host_prep(X, Ws, bWs, a1, a2, ba):
    """float64 host precompute: per-layer A/B vectors, softmax denominators,
    packed device input arrays."""
    X64 = np.asarray(X).astype(np.float64)
    A_vecs, B_vecs, S = [], [], np.ones(N, dtype=np.float64)
    for l in range(L):
        W = np.asarray(Ws[l]).astype(np.float64)
        c1 = W.T @ np.asarray(a1[l]).astype(np.float64)
        c2 = W.T @ np.asarray(a2[l]).astype(np.float64)
        d1 = np.asarray(bWs[l]).astype(np.float64) @ np.asarray(a1[l]).astype(np.float64)
        d2 = np.asarray(bWs[l]).astype(np.float64) @ np.asarray(a2[l]).astype(np.float64)
        A = X64 @ c1 + d1 + float(ba[l])   # i-side (all constants folded)
        B = X64 @ c2 + d2                  # j-side
        A_vecs.append(A)
        B_vecs.append(B)
        # S_l[i] = sum_j exp(leaky(A_i + B_j)) via sorted B:
        Bs = np.sort(B)
        suf_q = np.concatenate([np.cumsum(np.exp(Bs)[::-1])[::-1], [0.0]])
        pre_q5 = np.concatenate([[0.0], np.cumsum(np.exp(ALPHA * Bs))])
        k = np.searchsorted(Bs, -A, side="right")
        S *= np.exp(A) * suf_q[k] + np.exp(ALPHA * A) * pre_q5[k]
    scale = 1.0 / S

    bigW = np.zeros((8, N), dtype=BF16)
    for l in range(L):
        bh, bl = _split_bf16(B_vecs[l])
        bigW[2 * l] = bh
        bigW[2 * l + 1] = bl
    bigW[6] = BF16(1.0)
    bigW[7] = BF16(1.0)

    Xf = np.asarray(X).astype(np.float64)
    xh64 = Xf.astype(BF16).astype(np.float64)
    xh = Xf.astype(BF16)
    xl = (Xf - xh64).astype(BF16)

    def pack_x(xa):
        return np.ascontiguousarray(
            xa.reshape(NB, 128, D).transpose(1, 0, 2)).reshape(128, NB * D)

    xh_p, xl_p = pack_x(xh), pack_x(xl)

    rhsA_cores = []
    for core in range(N_CORES):
        r = np.zeros((8, L * NCHUNK * CHUNK), dtype=BF16)
        for l in range(L):
            ah, al = _split_bf16(A_vecs[l])
            for c in range(NCHUNK):
                lo = core * ROWS + c * CHUNK
                sl = slice((l * NCHUNK + c) * CHUNK,
                           (l * NCHUNK + c + 1) * CHUNK)
                r[2 * l, sl] = BF16(1.0)
                r[2 * l + 1, sl] = BF16(1.0)
                r[6, sl] = ah[lo:lo + CHUNK]
                r[7, sl] = al[lo:lo + CHUNK]
        rhsA_cores.append(r)
    return bigW, xh_p, xl_p, rhsA_cores, scale


def kernel(X, A, Ws, bWs, a1, a2, ba):
    from concourse.bass_utils import run_bass_kernel_spmd

    bigW, xh, xl, rhsA_cores, scale = _host_prep(X, Ws, bWs, a1, a2, ba)

    if "nc" not in _CACHED:
        _CACHED["nc"] = _build_nc(reps=1)
    nc = _CACHED["nc"]

    in_maps = [{"bigW": bigW, "xh": xh, "xl": xl, "rhsA": rhsA_cores[c]}
               for c in range(N_CORES)]
    res = run_bass_kernel_spmd(nc, in_maps, core_ids=list(range(N_CORES)))

    out = np.empty((N, D), dtype=np.float32)
    for c in range(N_CORES):
        yt = res.results[c]["yt"]          # [D, ROWS] unnormalized
        rows = slice(c * ROWS, (c + 1) * ROWS)
        out[rows] = (yt.T.astype(np.float64)
                     * scale[rows][:, None]).astype(np.float32)
    return out


# revision 19
# speedup vs baseline: 1.3206x; 1.3206x over previous
"""Trainium2 Bass kernel for nn_MAS (3-layer GAT-style attention product).

Math:
    for l in 0..2:
        Wh  = X @ Ws[l].T + bWs[l]
        e_ij = leaky_relu(f1_i + f2_j + ba[l]),  f1 = Wh@a1[l], f2 = Wh@a2[l]
        alpha = softmax_row(e);  A_MAS *= alpha
    out = A_MAS @ X

Key identities:
  * f1 = X @ (Ws.T a1) + bWs.a1 -> host computes per-layer vectors
    A_l[i] (= f1_i + consts) and B_l[j] (= f2_j) in float64.
  * prod_l softmax(e_l) = exp(sum_l leaky(z_l)) / prod_l S_l with
    z_l[i,j] = A_l[i] + B_l[j].  Row sums S_l are host-computable in
    O(N log N) by sorting B_l, so the device computes only
    YT = P^T-layout @ X with P = exp(sum_l leaky(z_l)) and the host applies
    the 1/prod_l S_l row scaling (plus the final transpose).

Device ([j, i] transposed layout -> no on-chip transposes):
  * PE: z_l tiles [128 j, 512 i] as K=8 bf16 outer-sum matmuls into PSUM
    (bf16 hi+lo splits of A/B make z exact to ~fp32).
  * ACT: Prelu(alpha=0.2) == exact leaky;  DVE: fused max(z,.2z)+m custom op;
    one Exp (ACT) since the product of exps is the exp of the summed leakys.
  * PE: YT[64, 512] accumulated over the 64 j-tiles, X in bf16 hi+lo splits.
Each of 8 cores handles 1024 i-rows; X replicated; no collectives.
"""

import sys

sys.path.insert(0, "/opt/trn_rl_repo")

from contextlib import ExitStack

import numpy as np
import ml_dtypes

BF16 = ml_dtypes.bfloat16

N = 8192
D = 64
L = 3
ALPHA = 0.2
N_CORES = 8
ROWS = N // N_CORES          # 1024 i-rows per core
CHUNK = 512                  # i-extent per chunk
NCHUNK = ROWS // CHUNK       # 2
NB = N // 128                # 64 j-tiles
GRP = 2                      # j-tiles per staging group (one per PE row strip;
                             # each strip's z lands in its own PSUM bank)
NGRP = NB // GRP             # 32

# Tuning knobs (read at _build_nc time).
CONF = {
    "use_custom_dve": True,
    # out of every 8 groups, this many use the DVE-heavy variant
    # (ACT: Prelu(z0)+Exp; DVE: leaky_add(z1), leaky_add(z2)); the rest use
    # (ACT: Prelu(z0), Prelu(z1), Exp; DVE/Pool: add, leaky_add(z2)).
    "beta8": 4,
    "pool_add": False,  # non-beta groups: do the m+t add on GPSIMD
    "mbufs": 4,
    "pbufs": 4,
    "zbufs": 3,
    # timing-only probes (break correctness):
    "probe_skip_las": False,
    "probe_skip_final": False,
}

_CACHED = {}


def _get_leaky_add():
    """Register (once) the fused DVE op  out = max(in0, alpha*in0) + in1."""
    if "leaky_add" in _CACHED:
        return _CACHED["leaky_add"]
    from concourse import dve_ops as dvo
    from concourse.dve_spec import Spec, Src0, Src1, C2, maxx, lower
    from concourse.dve_uop import DveOpSpec

    name = "LEAKY_ADD_MAS"
    spec = Spec(
        body=maxx(Src0, Src0 * C2) + Src1,
        reference=lambda in0, in1, imm2: np.maximum(in0, in0 * imm2) + in1,
    )
    shas = {}
    for ver in ("v3", "v4"):
        tmp = DveOpSpec(name=name, opcode=0, uops=lower(spec, ver=ver),
                        rd1_en=True)
        shas[ver] = tmp.sha(ver)
    op = dvo.DveOp(name, spec, subdim=False, uops_sha=shas)
    if name not in dvo._SUB_OPCODE_FOR_NAME:
        dvo.OPS.append(op)
        row = dvo._CUSTOM_DVE_ROW_BASE + len(dvo.OPS) - 1
        assert row < 0x20
        dvo._SUB_OPCODE_FOR_NAME[name] = row
    _CACHED["leaky_add"] = op
    return op


def _build_nc(reps: int = 1, dyn_loop: bool = False):
    import concourse.bass as bass
    import concourse.tile as tile
    from concourse import bacc, mybir

    nc = bacc.Bacc("TRN2", target_bir_lowering=False, debug=False,
                   num_devices=N_CORES)
    f32 = mybir.dt.float32
    bf16 = mybir.dt.bfloat16
    AF = mybir.ActivationFunctionType
    leaky_add = _get_leaky_add() if CONF["use_custom_dve"] else None
    if dyn_loop:
        nit_d = nc.dram_tensor("nit", [1, 1], mybir.dt.int32,
                               kind="ExternalInput")

    # K=8 outer-sum operands: lhsT rows (B0h,B0l,B1h,B1l,B2h,B2l,1,1);
    # rhs rows for layer l: rows 2l,2l+1 = ones, row 6/7 = Ah_l/Al_l, rest 0.
    bigW_d = nc.dram_tensor("bigW", [128, (NB // 2) * 128], bf16,
                            kind="ExternalInput")
    xhl_d = nc.dram_tensor("xhl", [128, NB * 2 * D], bf16,
                           kind="ExternalInput")
    rhsA_d = nc.dram_tensor("rhsA", [128, L * NCHUNK * CHUNK], bf16,
                            kind="ExternalInput")
    yt_d = nc.dram_tensor("yt", [2 * D, ROWS], f32, kind="ExternalOutput")

    with tile.TileContext(nc) as tc:
        with ExitStack() as ctx:
            consts = ctx.enter_context(tc.tile_pool(name="consts", bufs=1))
            zpool = ctx.enter_context(
                tc.tile_pool(name="zpool", bufs=CONF["zbufs"], space="PSUM"))
            opool = ctx.enter_context(
                tc.tile_pool(name="opool", bufs=2, space="PSUM"))
            mpool = ctx.enter_context(tc.tile_pool(name="mpool", bufs=CONF["mbufs"]))
            ppool = ctx.enter_context(tc.tile_pool(name="ppool", bufs=CONF["pbufs"]))

            sbW = consts.tile([128, (NB // 2) * 128], bf16)
            nc.sync.dma_start(sbW[:], bigW_d.ap()[:])
            sbA = consts.tile([128, L * NCHUNK * CHUNK], bf16)
            nc.sync.dma_start(sbA[:], rhsA_d.ap()[:])
            sbX = consts.tile([128, NB * 2 * D], bf16)
            nc.gpsimd.dma_start(sbX[:], xhl_d.ap()[:])
            ysb = consts.tile([2 * D, ROWS], f32)

            GW = GRP * CHUNK  # free-dim columns per staged group

            def emit_tail(pso, p, g):
                """Final matmuls for group g (deferred one group for
                ACT/PE pipelining)."""
                if CONF["probe_skip_final"]:
                    if g == NGRP - 1:
                        nc.tensor.matmul(pso[:], sbX[:, 0:2 * D],
                                         p[:, 0:CHUNK], start=True, stop=True,
                                         skip_group_check=True)
                    return
                for bi in range(GRP):
                    b = g * GRP + bi
                    rhs = p[:, bi * CHUNK:(bi + 1) * CHUNK]
                    nc.tensor.matmul(
                        pso[:], sbX[:, b * 2 * D:(b + 1) * 2 * D], rhs,
                        start=(b == 0), stop=(b == NB - 1),
                        skip_group_check=True)

            def emit_body(_rep):
                for c in range(NCHUNK):
                    pso = opool.tile([128, CHUNK], f32,
                                     name=f"pso_{_rep}_{c}", tag="pso")
                    pending = None
                    for g in range(NGRP):
                        beta = CONF["use_custom_dve"] and \
                            (g % 8) < CONF["beta8"]
                        zs = [zpool.tile([128, GW], f32,
                                         name=f"z_{_rep}_{c}_{g}_{l}",
                                         tag="z")
                              for l in range(L)]
                        for l in range(L):
                            for bi in range(GRP):
                                b = g * GRP + bi
                                s = 32 * bi
                                asl = slice((l * NCHUNK + c) * CHUNK,
                                            (l * NCHUNK + c + 1) * CHUNK)
                                nc.tensor.matmul(
                                    zs[l][:, bi * CHUNK:(bi + 1) * CHUNK],
                                    sbW[s:s + 8,
                                        (b // 2) * 128:(b // 2 + 1) * 128],
                                    sbA[s:s + 8, asl],
                                    start=True, stop=True,
                                    tile_position=(s, 0))
                        m = mpool.tile([128, GW], f32,
                                       name=f"m_{_rep}_{c}_{g}", tag="m")
                        nc.scalar.activation(m[:], zs[0][:], AF.Prelu,
                                             bias=0.0, scale=1.0, alpha=ALPHA)
                        if beta:
                            for l in (1, 2):
                                if not CONF["probe_skip_las"]:
                                    nc.vector._custom_dve(
                                        leaky_add, out=m[:], in0=zs[l][:],
                                        in1=m[:], imm2=ALPHA)
                        else:
                            t = mpool.tile([128, GW], f32,
                                           name=f"t_{_rep}_{c}_{g}", tag="t")
                            nc.scalar.activation(t[:], zs[1][:], AF.Prelu,
                                                 bias=0.0, scale=1.0,
                                                 alpha=ALPHA)
                            if CONF["pool_add"]:
                                nc.gpsimd.tensor_add(m[:], m[:], t[:])
                            else:
                                nc.vector.tensor_add(m[:], m[:], t[:])
                            if CONF["use_custom_dve"]:
                                nc.vector._custom_dve(
                                    leaky_add, out=m[:], in0=zs[2][:],
                                    in1=m[:], imm2=ALPHA)
                            else:
                                t2 = mpool.tile([128, GW], f32,
                                                name=f"t2_{_rep}_{c}_{g}",
                                                tag="t")
                                nc.scalar.activation(t2[:], zs[2][:],
                                                     AF.Prelu, bias=0.0,
                                                     scale=1.0, alpha=ALPHA)
                                nc.vector.tensor_add(m[:], m[:], t2[:])
                        if pending is not None:
                            pm, pg = pending
                            pp = ppool.tile([128, GW], bf16,
                                            name=f"p_{_rep}_{c}_{pg}", tag="p")
                            nc.scalar.activation(pp[:], pm[:], AF.Exp)
                            emit_tail(pso, pp, pg)
                        pending = (m, g)
                    pm, pg = pending
                    pp = ppool.tile([128, GW], bf16,
                                    name=f"p_{_rep}_{c}_{pg}", tag="p")
                    nc.scalar.activation(pp[:], pm[:], AF.Exp)
                    emit_tail(pso, pp, pg)
                    nc.scalar.copy(ysb[:, c * CHUNK:(c + 1) * CHUNK], pso[:])

            if dyn_loop:
                nit_sb = consts.tile([1, 1], mybir.dt.int32)
                nc.sync.dma_start(nit_sb[:], nit_d.ap()[:])
                nit = nc.values_load(
                    nit_sb[0:1, 0:1].to_broadcast((1, 1)))
                with tc.For_i(0, nit, 1,
                              hint_engines=(mybir.EngineType.PE,
                                            mybir.EngineType.Activation,
                                            mybir.EngineType.DVE)):
                    emit_body("dyn")
            else:
                for _rep in range(reps):
                    emit_body(_rep)
            nc.sync.dma_start(yt_d.ap()[:], ysb[:])
    nc.finalize()
    return nc


def _split_bf16(a):
    hi = a.astype(BF16)
    lo = (a - hi.astype(np.float64)).astype(BF16)
    return hi, lo


def _host_prep(X, Ws, bWs, a1, a2, ba):
    """float64 host precompute: per-layer A/B vectors, softmax denominators,
    packed device input arrays."""
    X64 = np.asarray(X).astype(np.float64)
    A_vecs, B_vecs, S = [], [], np.ones(N, dtype=np.float64)
    for l in range(L):
        W = np.asarray(Ws[l]).astype(np.float64)
        c1 = W.T @ np.asarray(a1[l]).astype(np.float64)
        c2 = W.T @ np.asarray(a2[l]).astype(np.float64)
        d1 = np.asarray(bWs[l]).astype(np.float64) @ np.asarray(a1[l]).astype(np.float64)
        d2 = np.asarray(bWs[l]).astype(np.float64) @ np.asarray(a2[l]).astype(np.float64)
        A = X64 @ c1 + d1 + float(ba[l])   # i-side (all constants folded)
        B = X64 @ c2 + d2                  # j-side
        A_vecs.append(A)
        B_vecs.append(B)
        # S_l[i] = sum_j exp(leaky(A_i + B_j)) via sorted B:
        Bs = np.sort(B)
        suf_q = np.concatenate([np.cumsum(np.exp(Bs)[::-1])[::-1], [0.0]])
        pre_q5 = np.concatenate([[0.0], np.cumsum(np.exp(ALPHA * Bs))])
        k = np.searchsorted(Bs, -A, side="right")
        S *= np.exp(A) * suf_q[k] + np.exp(ALPHA * A) * pre_q5[k]
    scale = 1.0 / S

    # z-gen lhsT blocks, strip-packed: W-block for j-tile b lives at
    # partitions 32*(b%4) .. +8, columns (b//4)*128 .. +128.
    wrows = np.zeros((8, N), dtype=BF16)
    for l in range(L):
        bh, bl = _split_bf16(B_vecs[l])
        wrows[2 * l] = bh
        wrows[2 * l + 1] = bl
    wrows[6] = BF16(1.0)
    wrows[7] = BF16(1.0)
    bigW = np.zeros((128, (NB // 2) * 128), dtype=BF16)
    for b in range(NB):
        s = 32 * (b % 2)
        bigW[s:s + 8, (b // 2) * 128:(b // 2 + 1) * 128] = \
            wrows[:, b * 128:(b + 1) * 128]

    # final-matmul lhsT: [Xh_b | Xl_b] stacked along free dim per j-tile.
    Xf = np.asarray(X).astype(np.float64)
    xh = Xf.astype(BF16)
    xl = (Xf - xh.astype(np.float64)).astype(BF16)
    xhl = np.empty((128, NB * 2 * D), dtype=BF16)
    for b in range(NB):
        xhl[:, b * 2 * D:b * 2 * D + D] = xh[b * 128:(b + 1) * 128, :]
        xhl[:, b * 2 * D + D:(b + 1) * 2 * D] = xl[b * 128:(b + 1) * 128, :]

    # z-gen rhs vectors, replicated at all four 32-partition strips.
    rhsA_cores = []
    for core in range(N_CORES):
        r = np.zeros((8, L * NCHUNK * CHUNK), dtype=BF16)
        for l in range(L):
            ah, al = _split_bf16(A_vecs[l])
            for c in range(NCHUNK):
                lo = core * ROWS + c * CHUNK
                sl = slice((l * NCHUNK + c) * CHUNK,
                           (l * NCHUNK + c + 1) * CHUNK)
                r[2 * l, sl] = BF16(1.0)
                r[2 * l + 1, sl] = BF16(1.0)
                r[6, sl] = ah[lo:lo + CHUNK]
                r[7, sl] = al[lo:lo + CHUNK]
        rr = np.zeros((128, L * NCHUNK * CHUNK), dtype=BF16)
        for s in range(2):
            rr[32 * s:32 * s + 8] = r
        rhsA_cores.append(rr)
    return bigW, xhl, rhsA_cores, scale


def kernel(X, A, Ws, bWs, a1, a2, ba):
    from concourse.bass_utils import run_bass_kernel_spmd

    bigW, xhl, rhsA_cores, scale = _host_prep(X, Ws, bWs, a1, a2, ba)

    if "nc" not in _CACHED:
        _CACHED["nc"] = _build_nc(reps=1)
    nc = _CACHED["nc"]

    in_maps = [{"bigW": bigW, "xhl": xhl, "rhsA": rhsA_cores[c]}
               for c in range(N_CORES)]
    res = run_bass_kernel_spmd(nc, in_maps, core_ids=list(range(N_CORES)))

    out = np.empty((N, D), dtype=np.float32)
    for c in range(N_CORES):
        yt = res.results[c]["yt"].astype(np.float64)  # [2D, ROWS], Yh|Yl
        y = yt[:D] + yt[D:]
        rows = slice(c * ROWS, (c + 1) * ROWS)
        out[rows] = (y.T * scale[rows][:, None]).astype(np.float32)
    return out


# revision 22
# speedup vs baseline: 1.3379x; 1.0131x over previous
"""Trainium2 Bass kernel for nn_MAS (3-layer GAT-style attention product).

Math:
    for l in 0..2:
        Wh  = X @ Ws[l].T + bWs[l]
        e_ij = leaky_relu(f1_i + f2_j + ba[l]),  f1 = Wh@a1[l], f2 = Wh@a2[l]
        alpha = softmax_row(e);  A_MAS *= alpha
    out = A_MAS @ X

Key identities:
  * f1 = X @ (Ws.T a1) + bWs.a1 -> host computes per-layer vectors
    A_l[i] (= f1_i + consts) and B_l[j] (= f2_j) in float64.
  * prod_l softmax(e_l) = exp(sum_l leaky(z_l)) / prod_l S_l with
    z_l[i,j] = A_l[i] + B_l[j].  Row sums S_l are host-computable in
    O(N log N) by sorting B_l, so the device computes only
    YT = P^T-layout @ X with P = exp(sum_l leaky(z_l)) and the host applies
    the 1/prod_l S_l row scaling (plus the final transpose).

Device ([j, i] transposed layout -> no on-chip transposes):
  * PE: z_l tiles [128 j, 512 i] via K=8 bf16 outer-sum matmuls into PSUM
    (bf16 hi+lo splits of A/B keep z exact to ~fp32; two j-tiles packed into
    PE row strips 0/32 with tile_position so their matmuls run concurrently).
  * ACT: Prelu(alpha=0.2) == exact leaky; DVE: custom fused max(z,.2z)+m op
    (one Exp per group suffices since prod_l exp(m_l) = exp(sum_l m_l));
    the Exp + final matmuls are software-pipelined two groups behind the
    leaky chain to keep the ACT/PE FIFOs from head-of-line blocking.
  * PE: final matmuls use a stacked [Xh_b | Xl_b] lhsT (M=128, one matmul
    per j-tile); the Yh/Yl halves land on partitions 0-63/64-127 and are
    summed on the host.
Each of 8 cores handles 1024 i-rows; X replicated; no collectives.
"""

import sys

sys.path.insert(0, "/opt/trn_rl_repo")

from contextlib import ExitStack

import numpy as np
import ml_dtypes

BF16 = ml_dtypes.bfloat16

N = 8192
D = 64
L = 3
ALPHA = 0.2
N_CORES = 8
ROWS = N // N_CORES          # 1024 i-rows per core
CHUNK = 512                  # i-extent per chunk
NCHUNK = ROWS // CHUNK       # 2
NB = N // 128                # 64 j-tiles
GRP = 2                      # j-tiles per staging group (one per PE row strip;
                             # each strip's z lands in its own PSUM bank)
NGRP = NB // GRP             # 32

# Tuning knobs (read at _build_nc time).
CONF = {
    "use_custom_dve": True,
    # out of every 8 groups, this many use the DVE-heavy variant
    # (ACT: Prelu(z0)+Exp; DVE: leaky_add(z1), leaky_add(z2)); the rest use
    # (ACT: Prelu(z0), Prelu(z1), Exp; DVE/Pool: add, leaky_add(z2)).
    "beta8": 4,
    "pool_add": False,  # non-beta groups: do the m+t add on GPSIMD
    "mbufs": 6,
    "pbufs": 6,
    "zbufs": 3,
    "defer": 2,
    # timing-only probes (break correctness):
    "probe_skip_las": False,
    "probe_skip_final": False,
}

_CACHED = {}


def _get_leaky_add():
    """Register (once) the fused DVE op  out = max(in0, alpha*in0) + in1."""
    if "leaky_add" in _CACHED:
        return _CACHED["leaky_add"]
    from concourse import dve_ops as dvo
    from concourse.dve_spec import Spec, Src0, Src1, C2, maxx, lower
    from concourse.dve_uop import DveOpSpec

    name = "LEAKY_ADD_MAS"
    spec = Spec(
        body=maxx(Src0, Src0 * C2) + Src1,
        reference=lambda in0, in1, imm2: np.maximum(in0, in0 * imm2) + in1,
    )
    shas = {}
    for ver in ("v3", "v4"):
        tmp = DveOpSpec(name=name, opcode=0, uops=lower(spec, ver=ver),
                        rd1_en=True)
        shas[ver] = tmp.sha(ver)
    op = dvo.DveOp(name, spec, subdim=False, uops_sha=shas)
    if name not in dvo._SUB_OPCODE_FOR_NAME:
        dvo.OPS.append(op)
        row = dvo._CUSTOM_DVE_ROW_BASE + len(dvo.OPS) - 1
        assert row < 0x20
        dvo._SUB_OPCODE_FOR_NAME[name] = row
    _CACHED["leaky_add"] = op
    return op


def _build_nc(reps: int = 1, dyn_loop: bool = False):
    import concourse.bass as bass
    import concourse.tile as tile
    from concourse import bacc, mybir

    nc = bacc.Bacc("TRN2", target_bir_lowering=False, debug=False,
                   num_devices=N_CORES)
    f32 = mybir.dt.float32
    bf16 = mybir.dt.bfloat16
    AF = mybir.ActivationFunctionType
    leaky_add = _get_leaky_add() if CONF["use_custom_dve"] else None
    if dyn_loop:
        nit_d = nc.dram_tensor("nit", [1, 1], mybir.dt.int32,
                               kind="ExternalInput")

    # K=8 outer-sum operands: lhsT rows (B0h,B0l,B1h,B1l,B2h,B2l,1,1);
    # rhs rows for layer l: rows 2l,2l+1 = ones, row 6/7 = Ah_l/Al_l, rest 0.
    bigW_d = nc.dram_tensor("bigW", [128, (NB // 2) * 128], bf16,
                            kind="ExternalInput")
    xhl_d = nc.dram_tensor("xhl", [128, NB * 2 * D], bf16,
                           kind="ExternalInput")
    rhsA_d = nc.dram_tensor("rhsA", [128, L * NCHUNK * CHUNK], bf16,
                            kind="ExternalInput")
    yt_d = nc.dram_tensor("yt", [2 * D, ROWS], f32, kind="ExternalOutput")

    with tile.TileContext(nc) as tc:
        with ExitStack() as ctx:
            consts = ctx.enter_context(tc.tile_pool(name="consts", bufs=1))
            zpool = ctx.enter_context(
                tc.tile_pool(name="zpool", bufs=CONF["zbufs"], space="PSUM"))
            opool = ctx.enter_context(
                tc.tile_pool(name="opool", bufs=2, space="PSUM"))
            mpool = ctx.enter_context(tc.tile_pool(name="mpool", bufs=CONF["mbufs"]))
            ppool = ctx.enter_context(tc.tile_pool(name="ppool", bufs=CONF["pbufs"]))

            sbW = consts.tile([128, (NB // 2) * 128], bf16)
            nc.sync.dma_start(sbW[:], bigW_d.ap()[:])
            sbA = consts.tile([128, L * NCHUNK * CHUNK], bf16)
            nc.sync.dma_start(sbA[:], rhsA_d.ap()[:])
            sbX = consts.tile([128, NB * 2 * D], bf16)
            nc.gpsimd.dma_start(sbX[:], xhl_d.ap()[:])
            ysb = consts.tile([2 * D, ROWS], f32)

            GW = GRP * CHUNK  # free-dim columns per staged group

            def emit_tail(pso, p, g):
                """Final matmuls for group g (deferred one group for
                ACT/PE pipelining)."""
                if CONF["probe_skip_final"]:
                    if g == NGRP - 1:
                        nc.tensor.matmul(pso[:], sbX[:, 0:2 * D],
                                         p[:, 0:CHUNK], start=True, stop=True,
                                         skip_group_check=True)
                    return
                for bi in range(GRP):
                    b = g * GRP + bi
                    rhs = p[:, bi * CHUNK:(bi + 1) * CHUNK]
                    nc.tensor.matmul(
                        pso[:], sbX[:, b * 2 * D:(b + 1) * 2 * D], rhs,
                        start=(b == 0), stop=(b == NB - 1),
                        skip_group_check=True)

            def emit_body(_rep):
                for c in range(NCHUNK):
                    pso = opool.tile([128, CHUNK], f32,
                                     name=f"pso_{_rep}_{c}", tag="pso")
                    pending = []
                    for g in range(NGRP):
                        beta = CONF["use_custom_dve"] and \
                            (g % 8) < CONF["beta8"]
                        zs = [zpool.tile([128, GW], f32,
                                         name=f"z_{_rep}_{c}_{g}_{l}",
                                         tag="z")
                              for l in range(L)]
                        for l in range(L):
                            for bi in range(GRP):
                                b = g * GRP + bi
                                s = 32 * bi
                                asl = slice((l * NCHUNK + c) * CHUNK,
                                            (l * NCHUNK + c + 1) * CHUNK)
                                nc.tensor.matmul(
                                    zs[l][:, bi * CHUNK:(bi + 1) * CHUNK],
                                    sbW[s:s + 8,
                                        (b // 2) * 128:(b // 2 + 1) * 128],
                                    sbA[s:s + 8, asl],
                                    start=True, stop=True,
                                    tile_position=(s, 0))
                        m = mpool.tile([128, GW], f32,
                                       name=f"m_{_rep}_{c}_{g}", tag="m")
                        nc.scalar.activation(m[:], zs[0][:], AF.Prelu,
                                             bias=0.0, scale=1.0, alpha=ALPHA)
                        if beta:
                            for l in (1, 2):
                                if not CONF["probe_skip_las"]:
                                    nc.vector._custom_dve(
                                        leaky_add, out=m[:], in0=zs[l][:],
                                        in1=m[:], imm2=ALPHA)
                        else:
                            t = mpool.tile([128, GW], f32,
                                           name=f"t_{_rep}_{c}_{g}", tag="t")
                            nc.scalar.activation(t[:], zs[1][:], AF.Prelu,
                                                 bias=0.0, scale=1.0,
                                                 alpha=ALPHA)
                            if CONF["pool_add"]:
                                nc.gpsimd.tensor_add(m[:], m[:], t[:])
                            else:
                                nc.vector.tensor_add(m[:], m[:], t[:])
                            if CONF["use_custom_dve"]:
                                nc.vector._custom_dve(
                                    leaky_add, out=m[:], in0=zs[2][:],
                                    in1=m[:], imm2=ALPHA)
                            else:
                                t2 = mpool.tile([128, GW], f32,
                                                name=f"t2_{_rep}_{c}_{g}",
                                                tag="t")
                                nc.scalar.activation(t2[:], zs[2][:],
                                                     AF.Prelu, bias=0.0,
                                                     scale=1.0, alpha=ALPHA)
                                nc.vector.tensor_add(m[:], m[:], t2[:])
                        if len(pending) >= CONF["defer"]:
                            pm, pg = pending.pop(0)
                            pp = ppool.tile([128, GW], bf16,
                                            name=f"p_{_rep}_{c}_{pg}", tag="p")
                            nc.scalar.activation(pp[:], pm[:], AF.Exp)
                            emit_tail(pso, pp, pg)
                        pending.append((m, g))
                    while pending:
                        pm, pg = pending.pop(0)
                        pp = ppool.tile([128, GW], bf16,
                                        name=f"p_{_rep}_{c}_{pg}", tag="p")
                        nc.scalar.activation(pp[:], pm[:], AF.Exp)
                        emit_tail(pso, pp, pg)
                    nc.scalar.copy(ysb[:, c * CHUNK:(c + 1) * CHUNK], pso[:])

            if dyn_loop:
                nit_sb = consts.tile([1, 1], mybir.dt.int32)
                nc.sync.dma_start(nit_sb[:], nit_d.ap()[:])
                nit = nc.values_load(
                    nit_sb[0:1, 0:1].to_broadcast((1, 1)))
                with tc.For_i(0, nit, 1,
                              hint_engines=(mybir.EngineType.PE,
                                            mybir.EngineType.Activation,
                                            mybir.EngineType.DVE)):
                    emit_body("dyn")
            else:
                for _rep in range(reps):
                    emit_body(_rep)
            nc.sync.dma_start(yt_d.ap()[:], ysb[:])
    nc.finalize()
    return nc


def _split_bf16(a):
    hi = a.astype(BF16)
    lo = (a - hi.astype(np.float64)).astype(BF16)
    return hi, lo


def _host_prep(X, Ws, bWs, a1, a2, ba):
    """float64 host precompute: per-layer A/B vectors, softmax denominators,
    packed device input arrays."""
    X64 = np.asarray(X).astype(np.float64)
    A_vecs, B_vecs, S = [], [], np.ones(N, dtype=np.float64)
    for l in range(L):
        W = np.asarray(Ws[l]).astype(np.float64)
        c1 = W.T @ np.asarray(a1[l]).astype(np.float64)
        c2 = W.T @ np.asarray(a2[l]).astype(np.float64)
        d1 = np.asarray(bWs[l]).astype(np.float64) @ np.asarray(a1[l]).astype(np.float64)
        d2 = np.asarray(bWs[l]).astype(np.float64) @ np.asarray(a2[l]).astype(np.float64)
        A = X64 @ c1 + d1 + float(ba[l])   # i-side (all constants folded)
        B = X64 @ c2 + d2                  # j-side
        A_vecs.append(A)
        B_vecs.append(B)
        # S_l[i] = sum_j exp(leaky(A_i + B_j)) via sorted B:
        Bs = np.sort(B)
        suf_q = np.concatenate([np.cumsum(np.exp(Bs)[::-1])[::-1], [0.0]])
        pre_q5 = np.concatenate([[0.0], np.cumsum(np.exp(ALPHA * Bs))])
        k = np.searchsorted(Bs, -A, side="right")
        S *= np.exp(A) * suf_q[k] + np.exp(ALPHA * A) * pre_q5[k]
    scale = 1.0 / S

    # z-gen lhsT blocks, strip-packed: W-block for j-tile b lives at
    # partitions 32*(b%4) .. +8, columns (b//4)*128 .. +128.
    wrows = np.zeros((8, N), dtype=BF16)
    for l in range(L):
        bh, bl = _split_bf16(B_vecs[l])
        wrows[2 * l] = bh
        wrows[2 * l + 1] = bl
    wrows[6] = BF16(1.0)
    wrows[7] = BF16(1.0)
    bigW = np.zeros((128, (NB // 2) * 128), dtype=BF16)
    for b in range(NB):
        s = 32 * (b % 2)
        bigW[s:s + 8, (b // 2) * 128:(b // 2 + 1) * 128] = \
            wrows[:, b * 128:(b + 1) * 128]

    # final-matmul lhsT: [Xh_b | Xl_b] stacked along free dim per j-tile.
    Xf = np.asarray(X).astype(np.float64)
    xh = Xf.astype(BF16)
    xl = (Xf - xh.astype(np.float64)).astype(BF16)
    xhl = np.empty((128, NB * 2 * D), dtype=BF16)
    for b in range(NB):
        xhl[:, b * 2 * D:b * 2 * D + D] = xh[b * 128:(b + 1) * 128, :]
        xhl[:, b * 2 * D + D:(b + 1) * 2 * D] = xl[b * 128:(b + 1) * 128, :]

    # z-gen rhs vectors, replicated at all four 32-partition strips.
    rhsA_cores = []
    for core in range(N_CORES):
        r = np.zeros((8, L * NCHUNK * CHUNK), dtype=BF16)
        for l in range(L):
            ah, al = _split_bf16(A_vecs[l])
            for c in range(NCHUNK):
                lo = core * ROWS + c * CHUNK
                sl = slice((l * NCHUNK + c) * CHUNK,
                           (l * NCHUNK + c + 1) * CHUNK)
                r[2 * l, sl] = BF16(1.0)
                r[2 * l + 1, sl] = BF16(1.0)
                r[6, sl] = ah[lo:lo + CHUNK]
                r[7, sl] = al[lo:lo + CHUNK]
        rr = np.zeros((128, L * NCHUNK * CHUNK), dtype=BF16)
        for s in range(2):
            rr[32 * s:32 * s + 8] = r
        rhsA_cores.append(rr)
    return bigW, xhl, rhsA_cores, scale


def kernel(X, A, Ws, bWs, a1, a2, ba):
    from concourse.bass_utils import run_bass_kernel_spmd

    bigW, xhl, rhsA_cores, scale = _host_prep(X, Ws, bWs, a1, a2, ba)

    if "nc" not in _CACHED:
        _CACHED["nc"] = _build_nc(reps=1)
    nc = _CACHED["nc"]

    in_maps = [{"bigW": bigW, "xhl": xhl, "rhsA": rhsA_cores[c]}
               for c in range(N_CORES)]
    res = run_bass_kernel_spmd(nc, in_maps, core_ids=list(range(N_CORES)))

    out = np.empty((N, D), dtype=np.float32)
    for c in range(N_CORES):
        yt = res.results[c]["yt"].astype(np.float64)  # [2D, ROWS], Yh|Yl
        y = yt[:D] + yt[D:]
        rows = slice(c * ROWS, (c + 1) * ROWS)
        out[rows] = (y.T * scale[rows][:, None]).astype(np.float32)
    return out


# revision 23
# speedup vs baseline: 1.4057x; 1.0507x over previous
"""Trainium2 Bass kernel for nn_MAS (3-layer GAT-style attention product).

Math:
    for l in 0..2:
        Wh  = X @ Ws[l].T + bWs[l]
        e_ij = leaky_relu(f1_i + f2_j + ba[l]),  f1 = Wh@a1[l], f2 = Wh@a2[l]
        alpha = softmax_row(e);  A_MAS *= alpha
    out = A_MAS @ X

Key identities:
  * f1 = X @ (Ws.T a1) + bWs.a1 -> host computes per-layer vectors
    A_l[i] (= f1_i + consts) and B_l[j] (= f2_j) in float64.
  * prod_l softmax(e_l) = exp(sum_l leaky(z_l)) / prod_l S_l with
    z_l[i,j] = A_l[i] + B_l[j].  Row sums S_l are host-computable in
    O(N log N) by sorting B_l, so the device computes only
    YT = P^T-layout @ X with P = exp(sum_l leaky(z_l)) and the host applies
    the 1/prod_l S_l row scaling (plus the final transpose).

Device ([j, i] transposed layout -> no on-chip transposes):
  * PE: z_l tiles [128 j, 512 i] via K=8 bf16 outer-sum matmuls into PSUM
    (bf16 hi+lo splits of A/B keep z exact to ~fp32; two j-tiles packed into
    PE row strips 0/32 with tile_position so their matmuls run concurrently).
  * ACT: Prelu(alpha=0.2) == exact leaky; DVE: custom fused max(z,.2z)+m op
    (one Exp per group suffices since prod_l exp(m_l) = exp(sum_l m_l));
    the Exp + final matmuls are software-pipelined two groups behind the
    leaky chain to keep the ACT/PE FIFOs from head-of-line blocking.
  * PE: final matmuls use a stacked [Xh_b | Xl_b] lhsT (M=128, one matmul
    per j-tile); the Yh/Yl halves land on partitions 0-63/64-127 and are
    summed on the host.
Each of 8 cores handles 1024 i-rows; X replicated; no collectives.
"""

import sys

sys.path.insert(0, "/opt/trn_rl_repo")

from contextlib import ExitStack

import numpy as np
import ml_dtypes

BF16 = ml_dtypes.bfloat16

N = 8192
D = 64
L = 3
ALPHA = 0.2
N_CORES = 8
ROWS = N // N_CORES          # 1024 i-rows per core
CHUNK = 512                  # i-extent per chunk
NCHUNK = ROWS // CHUNK       # 2
NB = N // 128                # 64 j-tiles
GRP = 2                      # j-tiles per staging group (one per PE row strip;
                             # each strip's z lands in its own PSUM bank)
NGRP = NB // GRP             # 32

# Tuning knobs (read at _build_nc time).
CONF = {
    "use_custom_dve": True,
    # out of every 8 groups, this many use the DVE-heavy variant
    # (ACT: Prelu(z0)+Exp; DVE: leaky_add(z1), leaky_add(z2)); the rest use
    # (ACT: Prelu(z0), Prelu(z1), Exp; DVE/Pool: add, leaky_add(z2)).
    "beta8": 4,
    "pool_add": False,  # non-beta groups: do the m+t add on GPSIMD
    "mbufs": 6,
    "pbufs": 6,
    "zbufs": 3,
    "defer": 2,
    # timing-only probes (break correctness):
    "probe_skip_las": False,
    "probe_skip_final": False,
}

_CACHED = {}


def _get_leaky_add():
    """Register (once) the fused DVE op  out = max(in0, alpha*in0) + in1."""
    if "leaky_add" in _CACHED:
        return _CACHED["leaky_add"]
    from concourse import dve_ops as dvo
    from concourse.dve_spec import Spec, Src0, Src1, C2, maxx, lower
    from concourse.dve_uop import DveOpSpec

    name = "LEAKY_ADD_MAS"
    spec = Spec(
        body=maxx(Src0, Src0 * C2) + Src1,
        reference=lambda in0, in1, imm2: np.maximum(in0, in0 * imm2) + in1,
    )
    shas = {}
    for ver in ("v3", "v4"):
        tmp = DveOpSpec(name=name, opcode=0, uops=lower(spec, ver=ver),
                        rd1_en=True)
        shas[ver] = tmp.sha(ver)
    op = dvo.DveOp(name, spec, subdim=False, uops_sha=shas)
    if name not in dvo._SUB_OPCODE_FOR_NAME:
        dvo.OPS.append(op)
        row = dvo._CUSTOM_DVE_ROW_BASE + len(dvo.OPS) - 1
        assert row < 0x20
        dvo._SUB_OPCODE_FOR_NAME[name] = row
    _CACHED["leaky_add"] = op
    return op


def _build_nc(reps: int = 1, dyn_loop: bool = False):
    import concourse.bass as bass
    import concourse.tile as tile
    from concourse import bacc, mybir

    nc = bacc.Bacc("TRN2", target_bir_lowering=False, debug=False,
                   num_devices=N_CORES)
    f32 = mybir.dt.float32
    bf16 = mybir.dt.bfloat16
    AF = mybir.ActivationFunctionType
    leaky_add = _get_leaky_add() if CONF["use_custom_dve"] else None
    if dyn_loop:
        nit_d = nc.dram_tensor("nit", [1, 1], mybir.dt.int32,
                               kind="ExternalInput")

    # K=8 outer-sum operands: lhsT rows (B0h,B0l,B1h,B1l,B2h,B2l,1,1);
    # rhs rows for layer l: rows 2l,2l+1 = ones, row 6/7 = Ah_l/Al_l, rest 0.
    bigW_d = nc.dram_tensor("bigW", [128, (NB // 2) * 128], bf16,
                            kind="ExternalInput")
    xhl_d = nc.dram_tensor("xhl", [128, NB * 2 * D], bf16,
                           kind="ExternalInput")
    rhsA_d = nc.dram_tensor("rhsA", [128, L * NCHUNK * CHUNK], bf16,
                            kind="ExternalInput")
    yt_d = nc.dram_tensor("yt", [2 * D, ROWS], f32, kind="ExternalOutput")

    with tile.TileContext(nc) as tc:
        with ExitStack() as ctx:
            consts = ctx.enter_context(tc.tile_pool(name="consts", bufs=1))
            zpool = ctx.enter_context(
                tc.tile_pool(name="zpool", bufs=CONF["zbufs"], space="PSUM"))
            opool = ctx.enter_context(
                tc.tile_pool(name="opool", bufs=2, space="PSUM"))
            mpool = ctx.enter_context(tc.tile_pool(name="mpool", bufs=CONF["mbufs"]))
            ppool = ctx.enter_context(tc.tile_pool(name="ppool", bufs=CONF["pbufs"]))

            sbW = consts.tile([128, (NB // 2) * 128], bf16)
            nc.sync.dma_start(sbW[:], bigW_d.ap()[:])
            sbA = consts.tile([128, L * NCHUNK * CHUNK], bf16)
            nc.sync.dma_start(sbA[:], rhsA_d.ap()[:])
            sbX = consts.tile([128, NB * 2 * D], bf16)
            xq = NB * 2 * D // 4
            for q in range(4):
                nc.gpsimd.dma_start(sbX[:, q * xq:(q + 1) * xq],
                                    xhl_d.ap()[:, q * xq:(q + 1) * xq])
            ysb = consts.tile([2 * D, ROWS], f32)

            GW = GRP * CHUNK  # free-dim columns per staged group

            def emit_tail(pso, p, g):
                """Final matmuls for group g (deferred one group for
                ACT/PE pipelining)."""
                if CONF["probe_skip_final"]:
                    if g == NGRP - 1:
                        nc.tensor.matmul(pso[:], sbX[:, 0:2 * D],
                                         p[:, 0:CHUNK], start=True, stop=True,
                                         skip_group_check=True)
                    return
                for bi in range(GRP):
                    b = g * GRP + bi
                    rhs = p[:, bi * CHUNK:(bi + 1) * CHUNK]
                    nc.tensor.matmul(
                        pso[:], sbX[:, b * 2 * D:(b + 1) * 2 * D], rhs,
                        start=(b == 0), stop=(b == NB - 1),
                        skip_group_check=True)

            def emit_body(_rep):
                for c in range(NCHUNK):
                    pso = opool.tile([128, CHUNK], f32,
                                     name=f"pso_{_rep}_{c}", tag="pso")
                    pending = []
                    for g in range(NGRP):
                        beta = CONF["use_custom_dve"] and \
                            (g % 8) < CONF["beta8"]
                        zs = [zpool.tile([128, GW], f32,
                                         name=f"z_{_rep}_{c}_{g}_{l}",
                                         tag="z")
                              for l in range(L)]
                        for l in range(L):
                            for bi in range(GRP):
                                b = g * GRP + bi
                                s = 32 * bi
                                asl = slice((l * NCHUNK + c) * CHUNK,
                                            (l * NCHUNK + c + 1) * CHUNK)
                                nc.tensor.matmul(
                                    zs[l][:, bi * CHUNK:(bi + 1) * CHUNK],
                                    sbW[s:s + 8,
                                        (b // 2) * 128:(b // 2 + 1) * 128],
                                    sbA[s:s + 8, asl],
                                    start=True, stop=True,
                                    tile_position=(s, 0))
                        m = mpool.tile([128, GW], f32,
                                       name=f"m_{_rep}_{c}_{g}", tag="m")
                        nc.scalar.activation(m[:], zs[0][:], AF.Prelu,
                                             bias=0.0, scale=1.0, alpha=ALPHA)
                        if beta:
                            for l in (1, 2):
                                if not CONF["probe_skip_las"]:
                                    nc.vector._custom_dve(
                                        leaky_add, out=m[:], in0=zs[l][:],
                                        in1=m[:], imm2=ALPHA)
                        else:
                            t = mpool.tile([128, GW], f32,
                                           name=f"t_{_rep}_{c}_{g}", tag="t")
                            nc.scalar.activation(t[:], zs[1][:], AF.Prelu,
                                                 bias=0.0, scale=1.0,
                                                 alpha=ALPHA)
                            if CONF["pool_add"]:
                                nc.gpsimd.tensor_add(m[:], m[:], t[:])
                            else:
                                nc.vector.tensor_add(m[:], m[:], t[:])
                            if CONF["use_custom_dve"]:
                                nc.vector._custom_dve(
                                    leaky_add, out=m[:], in0=zs[2][:],
                                    in1=m[:], imm2=ALPHA)
                            else:
                                t2 = mpool.tile([128, GW], f32,
                                                name=f"t2_{_rep}_{c}_{g}",
                                                tag="t")
                                nc.scalar.activation(t2[:], zs[2][:],
                                                     AF.Prelu, bias=0.0,
                                                     scale=1.0, alpha=ALPHA)
                                nc.vector.tensor_add(m[:], m[:], t2[:])
                        if len(pending) >= CONF["defer"]:
                            pm, pg = pending.pop(0)
                            pp = ppool.tile([128, GW], bf16,
                                            name=f"p_{_rep}_{c}_{pg}", tag="p")
                            nc.scalar.activation(pp[:], pm[:], AF.Exp)
                            emit_tail(pso, pp, pg)
                        pending.append((m, g))
                    while pending:
                        pm, pg = pending.pop(0)
                        pp = ppool.tile([128, GW], bf16,
                                        name=f"p_{_rep}_{c}_{pg}", tag="p")
                        nc.scalar.activation(pp[:], pm[:], AF.Exp)
                        emit_tail(pso, pp, pg)
                    nc.scalar.copy(ysb[:, c * CHUNK:(c + 1) * CHUNK], pso[:])

            if dyn_loop:
                nit_sb = consts.tile([1, 1], mybir.dt.int32)
                nc.sync.dma_start(nit_sb[:], nit_d.ap()[:])
                nit = nc.values_load(
                    nit_sb[0:1, 0:1].to_broadcast((1, 1)))
                with tc.For_i(0, nit, 1,
                              hint_engines=(mybir.EngineType.PE,
                                            mybir.EngineType.Activation,
                                            mybir.EngineType.DVE)):
                    emit_body("dyn")
            else:
                for _rep in range(reps):
                    emit_body(_rep)
            nc.sync.dma_start(yt_d.ap()[:], ysb[:])
    nc.finalize()
    return nc


def _split_bf16(a):
    hi = a.astype(BF16)
    lo = (a - hi.astype(np.float64)).astype(BF16)
    return hi, lo


def _host_prep(X, Ws, bWs, a1, a2, ba):
    """float64 host precompute: per-layer A/B vectors, softmax denominators,
    packed device input arrays."""
    X64 = np.asarray(X).astype(np.float64)
    A_vecs, B_vecs, S = [], [], np.ones(N, dtype=np.float64)
    for l in range(L):
        W = np.asarray(Ws[l]).astype(np.float64)
        c1 = W.T @ np.asarray(a1[l]).astype(np.float64)
        c2 = W.T @ np.asarray(a2[l]).astype(np.float64)
        d1 = np.asarray(bWs[l]).astype(np.float64) @ np.asarray(a1[l]).astype(np.float64)
        d2 = np.asarray(bWs[l]).astype(np.float64) @ np.asarray(a2[l]).astype(np.float64)
        A = X64 @ c1 + d1 + float(ba[l])   # i-side (all constants folded)
        B = X64 @ c2 + d2                  # j-side
        A_vecs.append(A)
        B_vecs.append(B)
        # S_l[i] = sum_j exp(leaky(A_i + B_j)) via sorted B:
        Bs = np.sort(B)
        suf_q = np.concatenate([np.cumsum(np.exp(Bs)[::-1])[::-1], [0.0]])
        pre_q5 = np.concatenate([[0.0], np.cumsum(np.exp(ALPHA * Bs))])
        k = np.searchsorted(Bs, -A, side="right")
        S *= np.exp(A) * suf_q[k] + np.exp(ALPHA * A) * pre_q5[k]
    scale = 1.0 / S

    # z-gen lhsT blocks, strip-packed: W-block for j-tile b lives at
    # partitions 32*(b%4) .. +8, columns (b//4)*128 .. +128.
    wrows = np.zeros((8, N), dtype=BF16)
    for l in range(L):
        bh, bl = _split_bf16(B_vecs[l])
        wrows[2 * l] = bh
        wrows[2 * l + 1] = bl
    wrows[6] = BF16(1.0)
    wrows[7] = BF16(1.0)
    bigW = np.zeros((128, (NB // 2) * 128), dtype=BF16)
    for b in range(NB):
        s = 32 * (b % 2)
        bigW[s:s + 8, (b // 2) * 128:(b // 2 + 1) * 128] = \
            wrows[:, b * 128:(b + 1) * 128]

    # final-matmul lhsT: [Xh_b | Xl_b] stacked along free dim per j-tile.
    Xf = np.asarray(X).astype(np.float64)
    xh = Xf.astype(BF16)
    xl = (Xf - xh.astype(np.float64)).astype(BF16)
    xhl = np.empty((128, NB * 2 * D), dtype=BF16)
    for b in range(NB):
        xhl[:, b * 2 * D:b * 2 * D + D] = xh[b * 128:(b + 1) * 128, :]
        xhl[:, b * 2 * D + D:(b + 1) * 2 * D] = xl[b * 128:(b + 1) * 128, :]

    # z-gen rhs vectors, replicated at all four 32-partition strips.
    rhsA_cores = []
    for core in range(N_CORES):
        r = np.zeros((8, L * NCHUNK * CHUNK), dtype=BF16)
        for l in range(L):
            ah, al = _split_bf16(A_vecs[l])
            for c in range(NCHUNK):
                lo = core * ROWS + c * CHUNK
                sl = slice((l * NCHUNK + c) * CHUNK,
                           (l * NCHUNK + c + 1) * CHUNK)
                r[2 * l, sl] = BF16(1.0)
                r[2 * l + 1, sl] = BF16(1.0)
                r[6, sl] = ah[lo:lo + CHUNK]
                r[7, sl] = al[lo:lo + CHUNK]
        rr = np.zeros((128, L * NCHUNK * CHUNK), dtype=BF16)
        for s in range(2):
            rr[32 * s:32 * s + 8] = r
        rhsA_cores.append(rr)
    return bigW, xhl, rhsA_cores, scale


def kernel(X, A, Ws, bWs, a1, a2, ba):
    from concourse.bass_utils import run_bass_kernel_spmd

    bigW, xhl, rhsA_cores, scale = _host_prep(X, Ws, bWs, a1, a2, ba)

    if "nc" not in _CACHED:
        _CACHED["nc"] = _build_nc(reps=1)
    nc = _CACHED["nc"]

    in_maps = [{"bigW": bigW, "xhl": xhl, "rhsA": rhsA_cores[c]}
               for c in range(N_CORES)]
    res = run_bass_kernel_spmd(nc, in_maps, core_ids=list(range(N_CORES)))

    out = np.empty((N, D), dtype=np.float32)
    for c in range(N_CORES):
        yt = res.results[c]["yt"].astype(np.float64)  # [2D, ROWS], Yh|Yl
        y = yt[:D] + yt[D:]
        rows = slice(c * ROWS, (c + 1) * ROWS)
        out[rows] = (y.T * scale[rows][:, None]).astype(np.float32)
    return out
